# revision 59
# baseline (speedup 1.0000x reference)
"""Trainium2 Bass kernel: Conv3d(3->24, k=3, valid) + bias -> min over depth -> softmax over channels.

Full inputs: x (128, 3, 16, 64, 64) f32, conv_weight (24, 3, 3, 3, 3), conv_bias (24,).
Output: (128, 24, 62, 62) f32.

Data-parallel over 8 cores (16 batch each). Per core (v2 design):
  Conv as TensorE matmul in fp16, TWO PSUM-accumulating passes per (h0, dt, unit)
  instead of three: pass A contracts K=126 partitions covering kw in {0,1} (two
  host-materialized w-shifted copies of the x window live in partition blocks
  0-62 / 63-125), pass B contracts K=63 (kw=2) reading block 0 at a +2w free
  offset.  M = 120 = 5 h-outputs x 24 out-chans, N = 496 = 31 w x 16 batch per
  unit (2 units cover w 0-61).  One 4-dim DMA per (h0, dt) slot on the SP HWDGE
  queue loads both blocks from a host tensor laid out (d, blk, h, c, w, b).
  Depth (14 dt) sequential; min over depth via pair-min ops (two PSUM tiles in,
  fp16 SBUF out) split across VectorE and GpSimd, then a 5-op fp16 merge tree
  on VectorE.  Epilogue: exp on ScalarE (per-partition bias), block-diag
  ones-matmul for softmax denominators, fast reciprocal + multiply on VectorE;
  output written in (h, o, w, b) layout and transposed back on host.
"""

import numpy as np

import concourse.bacc as bacc
import concourse.bass as bass
import concourse.mybir as mybir
import concourse.tile as tile
from concourse.bass_utils import run_bass_kernel_spmd

F32 = mybir.dt.float32
FP16 = mybir.dt.float16

B_CORE = 16
C_IN = 3
D_IN = 16
W_IN = 64
O = 24
D_OUT = 14
HW_OUT = 62
HB = [0, 5, 10, 15, 20, 25, 30, 35, 40, 45, 50, 55, 57]
NU = 496          # free size per unit: 31 w * 16 b
M_OUT = 120       # 5 hp * 24 o

_CACHE = {}


def build_host_tensors(conv_weight, conv_bias):
    """Weights packed for the two-pass layout.

    Partition order p = blk*63 + kd*21 + j*3 + c (blk only for pass A).
    lwA[p, hp*24+o] = W[o, c, kd, j-hp, kw=blk]; lwB covers kw=2.
    """
    lwA = np.zeros((126, M_OUT), np.float32)
    lwB = np.zeros((63, M_OUT), np.float32)
    for blk in range(2):
        for kd in range(3):
            for j in range(7):
                for c in range(C_IN):
                    p = kd * 21 + j * 3 + c
                    for hp in range(5):
                        kh = j - hp
                        if 0 <= kh <= 2:
                            m = hp * 24
                            lwA[blk * 63 + p, m:m + O] = conv_weight[:, c, kd, kh, blk]
                            if blk == 0:
                                lwB[p, m:m + O] = conv_weight[:, c, kd, kh, 2]
    ones = np.zeros((M_OUT, M_OUT), np.float32)
    for hp in range(5):
        ones[hp * 24:(hp + 1) * 24, hp * 24:(hp + 1) * 24] = 1.0
    biasv = np.zeros((M_OUT, 1), np.float32)
    for hp in range(5):
        biasv[hp * 24:(hp + 1) * 24, 0] = conv_bias
    return (lwA.astype(np.float16), lwB.astype(np.float16),
            ones.astype(np.float16), biasv)


def build_bass():
    nc = bacc.Bacc(None, target_bir_lowering=False)
    # x pre-arranged on host to (d, blk, h, c, w, b) fp16: blk 0 = x, blk 1 = x
    # shifted one step in w.  One slot DMA covers both kw partition blocks with
    # a 4-dim AP and 2KB contiguous (w, b) runs.
    x = nc.dram_tensor("x", [2, D_IN, 64, C_IN, 66, B_CORE], FP16,
                       kind="ExternalInput")
    lwA = nc.dram_tensor("lwA", [126, M_OUT], FP16, kind="ExternalInput")
    lwB = nc.dram_tensor("lwB", [63, M_OUT], FP16, kind="ExternalInput")
    ones = nc.dram_tensor("ones", [M_OUT, M_OUT], FP16, kind="ExternalInput")
    biasv = nc.dram_tensor("biasv", [M_OUT, 1], F32, kind="ExternalInput")
    y = nc.dram_tensor("y", [HW_OUT, O, HW_OUT, B_CORE], F32, kind="ExternalOutput")

    with tile.TileContext(nc) as tc:
        with (
            tc.tile_pool(name="const", bufs=1) as constp,
            tc.tile_pool(name="xs", bufs=28) as xsp,
            tc.tile_pool(name="mins", bufs=2) as minsp,
            tc.tile_pool(name="evt", bufs=2) as evp,
            tc.tile_pool(name="outp", bufs=2) as outp,
            tc.tile_pool(name="ps", bufs=4, space="PSUM") as psp,
        ):
            lwAt = constp.tile([128, M_OUT], FP16, tag="lwA")
            lwBt = constp.tile([128, M_OUT], FP16, tag="lwB")
            onest = constp.tile([128, M_OUT], FP16, tag="ones")
            biast = constp.tile([128, 1], F32, tag="bias")
            nc.sync.dma_start(lwAt[0:126, :], lwA[:, :])
            nc.sync.dma_start(lwBt[0:63, :], lwB[:, :])
            nc.scalar.dma_start(onest[0:M_OUT, :], ones[:, :])
            nc.scalar.dma_start(biast[0:M_OUT, :], biasv[:, :])

            pending = None
            # Epilogue v4: exp is monotone, so min(exp x) = exp(min x).
            # Act drains EVERY PSUM tile as an Exp-activation (same cost as
            # a copy, bias applied for free) into fp16 SBUF slots; DVE then
            # merges the 14 exp-domain partials with wide 2-byte 2x min ops.
            # PSUM views use rearrange+slice (dep-tracked bass APs, no raw
            # AP construction).  GPSIMD only issues output stores via SWDGE.
            for h0 in HB:
                mins = minsp.tile([128, 14 * 1024], FP16, tag="mins",
                                  name="mins")

                def sl(i, n=1024):
                    return mins[0:M_OUT, i * 1024:i * 1024 + n]

                for dt in range(D_OUT):
                    # slot DMA: (j c w b) merge into one contiguous 43KB run
                    # per (blk, kd) -> a 3-dim, 6-descriptor DMA
                    xt = xsp.tile([128, 1056], FP16, tag="xt", name="xt")
                    for blk in range(2):
                        srcap = x[blk:blk + 1, dt:dt + 3, h0:h0 + 7,
                                  :, :, :].rearrange(
                            "blk kd j c w b -> blk kd (j c) (w b)")
                        eng = nc.sync if blk == 0 else nc.scalar
                        eng.dma_start(xt[blk * 63:(blk + 1) * 63, :], srcap)
                    ps = psp.tile([128, 1024], F32, tag="ps", name="ps")
                    for u in range(2):
                        nc.tensor.matmul(
                            ps[0:M_OUT, u * 512:(u + 1) * 512],
                            lwAt[0:126, 0:M_OUT],
                            xt[0:126, u * 512:(u + 1) * 512],
                            start=True, stop=False)
                        nc.tensor.matmul(
                            ps[0:M_OUT, u * 512:(u + 1) * 512],
                            lwBt[0:63, 0:M_OUT],
                            xt[0:63, 32 + u * 512:32 + (u + 1) * 512],
                            start=False, stop=True)
                    nc.scalar.activation(sl(dt), ps[0:M_OUT, :],
                                         mybir.ActivationFunctionType.Exp,
                                         bias=biast[0:M_OUT, 0:1], scale=1.0)
                    # wide fp16 exp-domain min tree on DVE
                    if dt == 8:            # [s0..s3] ^= [s4..s7]
                        nc.vector.tensor_tensor(
                            sl(0, 4096), sl(0, 4096), sl(4, 4096),
                            mybir.AluOpType.min)
                    elif dt == 12:         # [s0..s3] ^= [s8..s11]
                        nc.vector.tensor_tensor(
                            sl(0, 4096), sl(0, 4096), sl(8, 4096),
                            mybir.AluOpType.min)
                    # deferred tail of the PREVIOUS block
                    if pending is not None:
                        ph0, pet, pot = pending
                        if dt == 7:
                            dps = psp.tile([128, 1024], F32, tag="ps",
                                           name="dps")
                            for u in range(2):
                                nc.tensor.matmul(
                                    dps[0:M_OUT, u * 512:(u + 1) * 512],
                                    onest[0:M_OUT, 0:M_OUT],
                                    pet[0:M_OUT, u * 512:(u + 1) * 512],
                                    start=True, stop=True)
                            dtmp = outp.tile([128, 1024], F32,
                                             tag="dtmp", name="dtmp")
                            nc.vector.tensor_scalar(
                                dtmp[0:M_OUT, :], dps[0:M_OUT, :],
                                0.0, None, mybir.AluOpType.bypass)
                            nc.vector.reciprocal_approx_fast(
                                pot[0:M_OUT, :], dtmp[0:M_OUT, :])
                            nc.vector.tensor_tensor(
                                pot[0:M_OUT, :], pet[0:M_OUT, :],
                                pot[0:M_OUT, :], mybir.AluOpType.mult)
                        elif dt == 9:
                            nc.gpsimd.dma_start(y[ph0:ph0 + 5, :, :, :],
                                                pot[0:M_OUT, 0:992])
                            pending = None
                # tail merges: [s0|s1] ^= [s2|s3]; s12 ^= s13; s0 ^= s1;
                # et = s0 ^ s12  (13 merges total in 6 wide DVE ops)
                nc.vector.tensor_tensor(
                    sl(0, 2048), sl(0, 2048), sl(2, 2048),
                    mybir.AluOpType.min)
                nc.vector.tensor_tensor(
                    sl(12), sl(12), sl(13), mybir.AluOpType.min)
                nc.vector.tensor_tensor(
                    sl(0), sl(0), sl(1), mybir.AluOpType.min)
                nc.vector.tensor_tensor(
                    sl(0), sl(0), sl(12), mybir.AluOpType.min)
                et = evp.tile([128, 1024], FP16, tag="et", name="et")
                nc.vector.tensor_scalar(et[0:M_OUT, :], sl(0), 0.0, None,
                                        mybir.AluOpType.bypass)
                ot = outp.tile([128, 1024], F32, tag="ot", name="ot")
                pending = (h0, et, ot)
            ph0, pet, pot = pending
            dps = psp.tile([128, 1024], F32, tag="ps", name="dps")
            for u in range(2):
                nc.tensor.matmul(dps[0:M_OUT, u * 512:(u + 1) * 512],
                                 onest[0:M_OUT, 0:M_OUT],
                                 pet[0:M_OUT, u * 512:(u + 1) * 512],
                                 start=True, stop=True)
            dtmp = outp.tile([128, 1024], F32, tag="dtmp", name="dtmp")
            nc.vector.tensor_scalar(dtmp[0:M_OUT, :], dps[0:M_OUT, :],
                                    0.0, None, mybir.AluOpType.bypass)
            nc.vector.reciprocal_approx_fast(
                pot[0:M_OUT, :], dtmp[0:M_OUT, :])
            nc.vector.tensor_tensor(pot[0:M_OUT, :], pet[0:M_OUT, :],
                                    pot[0:M_OUT, :], mybir.AluOpType.mult)
            nc.gpsimd.dma_start(y[ph0:ph0 + 5, :, :, :], pot[0:M_OUT, 0:992])
    nc.finalize()
    return nc


def kernel(x, conv_weight, conv_bias):
    x = np.asarray(x, dtype=np.float32)
    conv_weight = np.asarray(conv_weight, dtype=np.float32)
    conv_bias = np.asarray(conv_bias, dtype=np.float32)
    lwA, lwB, ones, biasv = build_host_tensors(conv_weight, conv_bias)
    if "nc" not in _CACHE:
        _CACHE["nc"] = build_bass()
    nc = _CACHE["nc"]
    core_ids = list(range(8))
    # (b, c, d, h, w) -> (d, h, c, w, b), then blk copies (w shift 0 / +1)
    x_t = np.transpose(x, (2, 3, 1, 4, 0)).astype(np.float16)  # d h c w b
    xr = np.zeros((2, D_IN, 64, C_IN, 66, 128), np.float16)
    xr[0, :, :, :, 0:64, :] = x_t
    xr[1, :, :, :, 0:63, :] = x_t[:, :, :, 1:64, :]
    in_maps = []
    for i in core_ids:
        in_maps.append({
            "x": np.ascontiguousarray(xr[:, :, :, :, :, i * B_CORE:(i + 1) * B_CORE]),  # blk d h c w b
            "lwA": lwA, "lwB": lwB, "ones": ones, "biasv": biasv,
        })
    res = run_bass_kernel_spmd(nc, in_maps, core_ids)
    # y per core: (h, o, w, b) -> (b, o, h, w)
    out = np.concatenate(
        [np.transpose(res.results[i]["y"], (3, 1, 0, 2)) for i in range(8)], axis=0)
    return np.ascontiguousarray(out)


if __name__ == "__main__":
    rng = np.random.default_rng(0)
    x = rng.standard_normal((128, 3, 16, 64, 64), dtype=np.float32)
    w = (rng.standard_normal((24, 3, 3, 3, 3)) * 0.1).astype(np.float32)
    b = (rng.standard_normal(24) * 0.1).astype(np.float32)
    out = kernel(x=x, conv_weight=w, conv_bias=b)
    print("out", out.shape, out.dtype)


# revision 60
# speedup vs baseline: 1.1200x; 1.1200x over previous
"""Trainium2 Bass kernel: Conv3d(3->24, k=3, valid) + bias -> min over depth -> softmax over channels.

Full inputs: x (128, 3, 16, 64, 64) f32, conv_weight (24, 3, 3, 3, 3), conv_bias (24,).
Output: (128, 24, 62, 62) f32.

Data-parallel over 8 cores (16 batch each). Per core (v2 design):
  Conv as TensorE matmul in fp16, TWO PSUM-accumulating passes per (h0, dt, unit)
  instead of three: pass A contracts K=126 partitions covering kw in {0,1} (two
  host-materialized w-shifted copies of the x window live in partition blocks
  0-62 / 63-125), pass B contracts K=63 (kw=2) reading block 0 at a +2w free
  offset.  M = 120 = 5 h-outputs x 24 out-chans, N = 496 = 31 w x 16 batch per
  unit (2 units cover w 0-61).  One 4-dim DMA per (h0, dt) slot on the SP HWDGE
  queue loads both blocks from a host tensor laid out (d, blk, h, c, w, b).
  Depth (14 dt) sequential; min over depth via pair-min ops (two PSUM tiles in,
  fp16 SBUF out) split across VectorE and GpSimd, then a 5-op fp16 merge tree
  on VectorE.  Epilogue: exp on ScalarE (per-partition bias), block-diag
  ones-matmul for softmax denominators, fast reciprocal + multiply on VectorE;
  output written in (h, o, w, b) layout and transposed back on host.
"""

import numpy as np

import concourse.bacc as bacc
import concourse.bass as bass
import concourse.mybir as mybir
import concourse.tile as tile
from concourse.bass_utils import run_bass_kernel_spmd

F32 = mybir.dt.float32
FP16 = mybir.dt.float16

B_CORE = 16
C_IN = 3
D_IN = 16
W_IN = 64
O = 24
D_OUT = 14
HW_OUT = 62
HB = [0, 5, 10, 15, 20, 25, 30, 35, 40, 45, 50, 55, 57]
NU = 496          # free size per unit: 31 w * 16 b
M_OUT = 120       # 5 hp * 24 o

_CACHE = {}


def build_host_tensors(conv_weight, conv_bias):
    """Weights packed for the two-pass layout.

    Partition order p = blk*63 + kd*21 + j*3 + c (blk only for pass A).
    lwA[p, hp*24+o] = W[o, c, kd, j-hp, kw=blk]; lwB covers kw=2.
    """
    lwA = np.zeros((126, M_OUT), np.float32)
    lwB = np.zeros((63, M_OUT), np.float32)
    for blk in range(2):
        for kd in range(3):
            for j in range(7):
                for c in range(C_IN):
                    p = kd * 21 + j * 3 + c
                    for hp in range(5):
                        kh = j - hp
                        if 0 <= kh <= 2:
                            m = hp * 24
                            lwA[blk * 63 + p, m:m + O] = conv_weight[:, c, kd, kh, blk]
                            if blk == 0:
                                lwB[p, m:m + O] = conv_weight[:, c, kd, kh, 2]
    ones = np.zeros((M_OUT, M_OUT), np.float32)
    for hp in range(5):
        ones[hp * 24:(hp + 1) * 24, hp * 24:(hp + 1) * 24] = 1.0
    biasv = np.zeros((M_OUT, 1), np.float32)
    for hp in range(5):
        biasv[hp * 24:(hp + 1) * 24, 0] = conv_bias
    return (lwA.astype(np.float16), lwB.astype(np.float16),
            ones.astype(np.float16), biasv)


def build_bass():
    nc = bacc.Bacc(None, target_bir_lowering=False)
    # x pre-arranged on host to (d, blk, h, c, w, b) fp16: blk 0 = x, blk 1 = x
    # shifted one step in w.  One slot DMA covers both kw partition blocks with
    # a 4-dim AP and 2KB contiguous (w, b) runs.
    x = nc.dram_tensor("x", [2, D_IN, 64, C_IN, 66, B_CORE], FP16,
                       kind="ExternalInput")
    lwA = nc.dram_tensor("lwA", [126, M_OUT], FP16, kind="ExternalInput")
    lwB = nc.dram_tensor("lwB", [63, M_OUT], FP16, kind="ExternalInput")
    ones = nc.dram_tensor("ones", [M_OUT, M_OUT], FP16, kind="ExternalInput")
    biasv = nc.dram_tensor("biasv", [M_OUT, 1], F32, kind="ExternalInput")
    y = nc.dram_tensor("y", [HW_OUT, O, HW_OUT, B_CORE], F32, kind="ExternalOutput")

    with tile.TileContext(nc) as tc:
        with (
            tc.tile_pool(name="const", bufs=1) as constp,
            tc.tile_pool(name="xs", bufs=28) as xsp,
            tc.tile_pool(name="mins", bufs=2) as minsp,
            tc.tile_pool(name="evt", bufs=2) as evp,
            tc.tile_pool(name="outp", bufs=2) as outp,
            tc.tile_pool(name="ps", bufs=4, space="PSUM") as psp,
        ):
            lwAt = constp.tile([128, M_OUT], FP16, tag="lwA")
            lwBt = constp.tile([128, M_OUT], FP16, tag="lwB")
            onest = constp.tile([128, M_OUT], FP16, tag="ones")
            biast = constp.tile([128, 1], F32, tag="bias")
            nc.sync.dma_start(lwAt[0:126, :], lwA[:, :])
            nc.sync.dma_start(lwBt[0:63, :], lwB[:, :])
            nc.scalar.dma_start(onest[0:M_OUT, :], ones[:, :])
            nc.scalar.dma_start(biast[0:M_OUT, :], biasv[:, :])

            pending = None
            # Epilogue v4: exp is monotone, so min(exp x) = exp(min x).
            # Act drains EVERY PSUM tile as an Exp-activation (same cost as
            # a copy, bias applied for free) into fp16 SBUF slots; DVE then
            # merges the 14 exp-domain partials with wide 2-byte 2x min ops.
            # PSUM views use rearrange+slice (dep-tracked bass APs, no raw
            # AP construction).  GPSIMD only issues output stores via SWDGE.
            for h0 in HB:
                mins = minsp.tile([128, 14 * 1024], FP16, tag="mins",
                                  name="mins")

                def sl(i, n=1024):
                    return mins[0:M_OUT, i * 1024:i * 1024 + n]

                for dt in range(D_OUT):
                    # slot DMA: (j c w b) merge into one contiguous 43KB run
                    # per (blk, kd) -> a 3-dim, 6-descriptor DMA
                    xt = xsp.tile([128, 1056], FP16, tag="xt", name="xt")
                    for blk in range(2):
                        srcap = x[blk:blk + 1, dt:dt + 3, h0:h0 + 7,
                                  :, :, :].rearrange(
                            "blk kd j c w b -> blk kd (j c) (w b)")
                        if blk == 0:
                            eng = nc.sync
                        else:
                            eng = nc.gpsimd if dt % 2 == 0 else nc.sync
                        eng.dma_start(xt[blk * 63:(blk + 1) * 63, :], srcap)
                    ps = psp.tile([128, 1024], F32, tag="ps", name="ps")
                    for u in range(2):
                        nc.tensor.matmul(
                            ps[0:M_OUT, u * 512:(u + 1) * 512],
                            lwAt[0:126, 0:M_OUT],
                            xt[0:126, u * 512:(u + 1) * 512],
                            start=True, stop=False)
                        nc.tensor.matmul(
                            ps[0:M_OUT, u * 512:(u + 1) * 512],
                            lwBt[0:63, 0:M_OUT],
                            xt[0:63, 32 + u * 512:32 + (u + 1) * 512],
                            start=False, stop=True)
                    nc.scalar.activation(sl(dt), ps[0:M_OUT, :],
                                         mybir.ActivationFunctionType.Exp,
                                         bias=biast[0:M_OUT, 0:1], scale=1.0)
                    # wide fp16 exp-domain min tree on DVE
                    if dt == 8:            # [s0..s3] ^= [s4..s7]
                        nc.vector.tensor_tensor(
                            sl(0, 4096), sl(0, 4096), sl(4, 4096),
                            mybir.AluOpType.min)
                    elif dt == 12:         # [s0..s3] ^= [s8..s11]
                        nc.vector.tensor_tensor(
                            sl(0, 4096), sl(0, 4096), sl(8, 4096),
                            mybir.AluOpType.min)
                    # deferred tail of the PREVIOUS block
                    if pending is not None:
                        ph0, pet, pot = pending
                        if dt == 7:
                            dps = psp.tile([128, 1024], F32, tag="ps",
                                           name="dps")
                            for u in range(2):
                                nc.tensor.matmul(
                                    dps[0:M_OUT, u * 512:(u + 1) * 512],
                                    onest[0:M_OUT, 0:M_OUT],
                                    pet[0:M_OUT, u * 512:(u + 1) * 512],
                                    start=True, stop=True)
                            dtmp = outp.tile([128, 1024], F32,
                                             tag="dtmp", name="dtmp")
                            nc.vector.tensor_scalar(
                                dtmp[0:M_OUT, :], dps[0:M_OUT, :],
                                0.0, None, mybir.AluOpType.bypass)
                            nc.vector.reciprocal_approx_fast(
                                pot[0:M_OUT, :], dtmp[0:M_OUT, :])
                            nc.vector.tensor_tensor(
                                pot[0:M_OUT, :], pet[0:M_OUT, :],
                                pot[0:M_OUT, :], mybir.AluOpType.mult)
                        elif dt == 9:
                            nc.gpsimd.dma_start(y[ph0:ph0 + 5, :, :, :],
                                                pot[0:M_OUT, 0:992])
                            pending = None
                # tail merges: [s0|s1] ^= [s2|s3]; s12 ^= s13; s0 ^= s1;
                # et = s0 ^ s12  (13 merges total in 6 wide DVE ops)
                nc.vector.tensor_tensor(
                    sl(0, 2048), sl(0, 2048), sl(2, 2048),
                    mybir.AluOpType.min)
                nc.vector.tensor_tensor(
                    sl(12), sl(12), sl(13), mybir.AluOpType.min)
                nc.vector.tensor_tensor(
                    sl(0), sl(0), sl(1), mybir.AluOpType.min)
                nc.vector.tensor_tensor(
                    sl(0), sl(0), sl(12), mybir.AluOpType.min)
                et = evp.tile([128, 1024], FP16, tag="et", name="et")
                nc.vector.tensor_scalar(et[0:M_OUT, :], sl(0), 0.0, None,
                                        mybir.AluOpType.bypass)
                ot = outp.tile([128, 1024], F32, tag="ot", name="ot")
                pending = (h0, et, ot)
            ph0, pet, pot = pending
            dps = psp.tile([128, 1024], F32, tag="ps", name="dps")
            for u in range(2):
                nc.tensor.matmul(dps[0:M_OUT, u * 512:(u + 1) * 512],
                                 onest[0:M_OUT, 0:M_OUT],
                                 pet[0:M_OUT, u * 512:(u + 1) * 512],
                                 start=True, stop=True)
            dtmp = outp.tile([128, 1024], F32, tag="dtmp", name="dtmp")
            nc.vector.tensor_scalar(dtmp[0:M_OUT, :], dps[0:M_OUT, :],
                                    0.0, None, mybir.AluOpType.bypass)
            nc.vector.reciprocal_approx_fast(
                pot[0:M_OUT, :], dtmp[0:M_OUT, :])
            nc.vector.tensor_tensor(pot[0:M_OUT, :], pet[0:M_OUT, :],
                                    pot[0:M_OUT, :], mybir.AluOpType.mult)
            nc.gpsimd.dma_start(y[ph0:ph0 + 5, :, :, :], pot[0:M_OUT, 0:992])
    nc.finalize()
    return nc


def kernel(x, conv_weight, conv_bias):
    x = np.asarray(x, dtype=np.float32)
    conv_weight = np.asarray(conv_weight, dtype=np.float32)
    conv_bias = np.asarray(conv_bias, dtype=np.float32)
    lwA, lwB, ones, biasv = build_host_tensors(conv_weight, conv_bias)
    if "nc" not in _CACHE:
        _CACHE["nc"] = build_bass()
    nc = _CACHE["nc"]
    core_ids = list(range(8))
    # (b, c, d, h, w) -> (d, h, c, w, b), then blk copies (w shift 0 / +1)
    x_t = np.transpose(x, (2, 3, 1, 4, 0)).astype(np.float16)  # d h c w b
    xr = np.zeros((2, D_IN, 64, C_IN, 66, 128), np.float16)
    xr[0, :, :, :, 0:64, :] = x_t
    xr[1, :, :, :, 0:63, :] = x_t[:, :, :, 1:64, :]
    in_maps = []
    for i in core_ids:
        in_maps.append({
            "x": np.ascontiguousarray(xr[:, :, :, :, :, i * B_CORE:(i + 1) * B_CORE]),  # blk d h c w b
            "lwA": lwA, "lwB": lwB, "ones": ones, "biasv": biasv,
        })
    res = run_bass_kernel_spmd(nc, in_maps, core_ids)
    # y per core: (h, o, w, b) -> (b, o, h, w)
    out = np.concatenate(
        [np.transpose(res.results[i]["y"], (3, 1, 0, 2)) for i in range(8)], axis=0)
    return np.ascontiguousarray(out)


if __name__ == "__main__":
    rng = np.random.default_rng(0)
    x = rng.standard_normal((128, 3, 16, 64, 64), dtype=np.float32)
    w = (rng.standard_normal((24, 3, 3, 3, 3)) * 0.1).astype(np.float32)
    b = (rng.standard_normal(24) * 0.1).astype(np.float32)
    out = kernel(x=x, conv_weight=w, conv_bias=b)
    print("out", out.shape, out.dtype)
